# revision 37
# baseline (speedup 1.0000x reference)
"""KNN classify kernel for TRN2 (8 NeuronCores).

Strategy: shard X over N (12500 points/core, padded to 12800). Scores are
computed as s[b,n] = 2*q.x with two fp8e4 DoubleRow matmuls (K_eff=256 each
over the 512 dims). The -||x||^2 term is handled structurally: the host
sorts each core's points by ||x||^2 and permutes columns so that each
pooling window (16 columns congruent mod 128 within a 2048-wide slab) holds
16 norm-consecutive points, dealt round-robin across slabs. PSUM tiles are
drained to SBUF fp16 (Scalar engine; DVE takes one chunk of every other
block to balance rates), then a DVE pairwise-max fold tree pools each slab
to 128 window maxima which are DMAd straight back to the host. The host
subtracts the per-window min-||x||^2, takes the global top-40 windows per
query, expands them (16 columns each), rescores exactly in fp64, takes
top-K and emits label votes.
"""

import sys

sys.path.insert(0, "/opt/trn_rl_repo")

import ml_dtypes
import numpy as np

import concourse.bacc as bacc
import concourse.mybir as mybir
from concourse import bass_utils
from concourse.tile import TileContext

B, D, N = 2048, 512, 100000
NCORES = 8
NSH = N // NCORES  # 12500
NPAD = 12800  # 25 * 512
P = 128
BLK = B // P  # 16 query blocks
NF = 512  # psum bank width (fp32 cols)
SLAB = 1024  # 2 psum banks
NFULL = 12  # full slabs; + 1 partial slab of 512
WPART = NPAD - NFULL * SLAB  # 512
NSLABS = NFULL + 1
RW = 16  # columns per pooled window
NWIN = SLAB // RW  # 64 windows per full slab
NWINP = WPART // RW  # 32 windows in partial slab
NFWIN = NFULL * NWIN  # 768 windows live in full slabs; the rest go to the partial
NPOOL = NFWIN + NWINP  # 800 pooled values per (core, query)
GB = 4  # query-block group size for batched folds

E4 = ml_dtypes.float8_e4m3
TOPW = 40  # windows expanded+rescored on host per query

_prog = None


def _build_program():
    nc = bacc.Bacc("TRN2", target_bir_lowering=False, debug=False, num_devices=NCORES)
    qt_d = nc.dram_tensor("qt", (BLK, P, 2, 2, P), mybir.dt.float8e4, kind="ExternalInput")
    xt_d = nc.dram_tensor("xt", (2, P, 2, NPAD), mybir.dt.float8e4, kind="ExternalInput")
    pool_d = nc.dram_tensor("pooled", (B, NPOOL), mybir.dt.float16, kind="ExternalOutput")

    DR = mybir.MatmulPerfMode.DoubleRow
    MAX = mybir.AluOpType.max

    pool_ap = pool_d.ap().rearrange("(blk p) w -> p blk w", p=P)

    with TileContext(nc) as tc:
        with (
            tc.tile_pool(name="const", bufs=1) as cpool,
            tc.tile_pool(name="scp", bufs=4) as scpool,
            tc.tile_pool(name="fp", bufs=3) as fpool,
            tc.tile_pool(name="psp", bufs=4, space="PSUM") as ppool,
        ):
            def load_qt(blk):
                t = cpool.tile([P, 2, 2, P], mybir.dt.float8e4, tag=f"qt{blk}", name=f"qt{blk}")
                nc.sync.dma_start(t, qt_d.ap()[blk])
                return t

            def load_xt(c, s):
                w = SLAB if s < NFULL else WPART
                t = cpool.tile([P, 2, w], mybir.dt.float8e4, tag=f"xt{c}_{s}", name=f"xt{c}_{s}")
                nc.sync.dma_start(t, xt_d.ap()[c][:, :, s * SLAB : s * SLAB + w])
                return t

            # first slab's inputs first so the PE can start ASAP
            qts = [load_qt(0)]
            xts = {(c, 0): load_xt(c, 0) for c in (0, 1)}
            qts += [load_qt(b) for b in range(1, BLK)]
            for s in range(1, NSLABS):
                for c in (0, 1):
                    xts[(c, s)] = load_xt(c, s)

            warm = cpool.tile([P, P], mybir.dt.float32, tag="warm", name="warm")
            nc.gpsimd.memset(warm, 0.0)

            # Warm-up: dummy matmuls with no DMA deps run during the initial
            # input-DMA wait so HAM un-throttles before real work.
            wps = ppool.tile([P, 2, NF], mybir.dt.float32, tag="ps", name="wps")
            for _ in range(12):
                nc.tensor.matmul(wps[:, 0, :P], warm, warm, start=True, stop=True)

            drain_ctr = 0
            for s in range(NSLABS):
                w = SLAB if s < NFULL else WPART
                nch = w // NF
                nwin = w // RW
                sct = "scg" if s < NFULL else "scgp"
                gb = GB
                for g in range(BLK // gb):
                    scg = scpool.tile([P, gb, nch, NF], mybir.dt.float16, tag=sct)
                    for b in range(gb):
                        blk = g * gb + b
                        ps = ppool.tile([P, 2, NF], mybir.dt.float32, tag="ps", name=f"ps{s}_{blk}")
                        for cpass in (0, 1):
                            for ch in range(nch):
                                nc.tensor.matmul(
                                    ps[:, ch, :],
                                    qts[blk][:, cpass],
                                    xts[(cpass, s)][:, :, ch * NF : (ch + 1) * NF],
                                    start=(cpass == 0),
                                    stop=(cpass == 1),
                                    perf_mode=DR,
                                )
                        if s < NFULL and drain_ctr % 8 == 7:
                            nc.vector.tensor_copy(out=scg[:, b], in_=ps[:, :nch, :])
                        else:
                            nc.scalar.copy(scg[:, b], ps[:, :nch, :])
                        drain_ctr += 1

                    # fold tree: pool columns mod (w//16) across the group
                    if s < NFULL:
                        f2 = fpool.tile([P, gb, NF], mybir.dt.float16, tag="f2")
                        nc.vector.tensor_tensor(
                            out=f2, in0=scg[:, :, 0, :], in1=scg[:, :, 1, :], op=MAX
                        )
                        prev, pw = f2, NF
                    else:
                        prev, pw = scg[:, :, 0, :], NF
                    lvl = 0
                    while pw > nwin:
                        pw //= 2
                        nxt = fpool.tile([P, gb, pw], mybir.dt.float16, tag=f"f{sct}{lvl}")
                        nc.vector.tensor_tensor(
                            out=nxt, in0=prev[:, :, :pw], in1=prev[:, :, pw:], op=MAX
                        )
                        prev = nxt
                        lvl += 1
                    nc.sync.dma_start(
                        pool_ap[:, g * gb : (g + 1) * gb, s * NWIN : s * NWIN + nwin],
                        prev,
                    )

    nc.compile()
    return nc


def _q8(a):
    return np.clip(a, -240.0, 240.0).astype(E4)


def _permutation():
    """sorted-rank r -> device column, for one core (NSH points).

    Window w = r//16 (16 norm-consecutive points); full-slab windows are
    dealt round-robin across the 6 full slabs; the remainder go to the
    partial slab. Returns (dev_col[r], pool_pos[w])."""
    r = np.arange(NSH)
    wnd = r // RW
    j = r % RW
    full = wnd < NFWIN
    s = wnd % NFULL
    k = wnd // NFULL
    col_full = s * SLAB + j * NWIN + k
    pk = wnd - NFWIN
    col_part = NFULL * SLAB + j * NWINP + pk
    dev_col = np.where(full, col_full, col_part)
    nwnd = (NSH + RW - 1) // RW
    wi = np.arange(nwnd)
    wfull = wi < NFWIN
    pool_pos = np.where(wfull, (wi % NFULL) * NWIN + wi // NFULL, NFWIN + (wi - NFWIN))
    return dev_col, pool_pos


def _prepare_inputs(queries, X):
    queries = np.asarray(queries, np.float32)
    X = np.asarray(X, np.float32)

    q8 = _q8(2.0 * queries)  # [B, D]
    # qt[blk, p, c, i, m] = q8[blk*128+m, c*256+i*128+p]
    qt = np.ascontiguousarray(q8.reshape(BLK, P, 2, 2, P).transpose(0, 4, 2, 3, 1))

    dev_col, pool_pos = _permutation()
    in_maps = []
    orig_maps = []
    cvecs = []
    for core in range(NCORES):
        sl = slice(core * NSH, (core + 1) * NSH)
        Xc = X[sl]
        x2 = (Xc.astype(np.float64) ** 2).sum(1)
        order = np.argsort(x2, kind="stable")  # ascending norm

        Xdev = np.zeros((NPAD, D), np.float32)
        Xdev[dev_col] = Xc[order]
        orig_of_col = np.full(NPAD, -1, np.int64)
        orig_of_col[dev_col] = core * NSH + order
        orig_maps.append(orig_of_col)

        x8 = _q8(Xdev)
        # xt[c, p, i, n] = x8[n, c*256+i*128+p]
        xt = np.ascontiguousarray(x8.reshape(NPAD, 2, 2, P).transpose(1, 3, 2, 0))

        # c[pool_pos] = min ||x||^2 of each window; +inf for empty windows
        x2s = x2[order]
        wmin = np.minimum.reduceat(x2s, np.arange(0, NSH, RW))
        cvec = np.full(NPOOL, 1e30, np.float64)
        cvec[pool_pos] = wmin
        cvecs.append(cvec)
        in_maps.append({"qt": qt, "xt": xt})
    return in_maps, orig_maps, cvecs


def _run_device(queries, X, trace=False, trace_kwargs=None):
    global _prog
    if _prog is None:
        _prog = _build_program()
    in_maps, orig_maps, cvecs = _prepare_inputs(queries, X)
    res = bass_utils.run_bass_kernel_spmd(
        _prog,
        in_maps,
        core_ids=list(range(NCORES)),
        trace=trace,
        **(trace_kwargs or {}),
    )
    res.orig_maps = orig_maps
    res.cvecs = cvecs
    return res


def _merge(queries, X, Y, K, res):
    pooled = np.stack(
        [res.results[c]["pooled"] for c in range(NCORES)]
    )  # [8, B, NPOOL] fp16
    orig = np.stack(res.orig_maps)  # [8, NPAD]
    cvec = np.stack(res.cvecs).astype(np.float32)  # [8, NPOOL]

    av = pooled.astype(np.float32) - cvec[:, None, :]
    av = av.transpose(1, 0, 2).reshape(B, NCORES * NPOOL)

    # pooled index -> (first device col, window step)
    pidx = np.arange(NPOOL)
    pfull = pidx < NFWIN
    col0_of = np.where(
        pfull, (pidx // NWIN) * SLAB + pidx % NWIN, NFULL * SLAB + (pidx - NFWIN)
    )
    step_of = np.where(pfull, NWIN, NWINP)
    col0 = np.tile(col0_of, NCORES)[None, :]
    wstep = np.tile(step_of, NCORES)[None, :]
    core_of = np.repeat(np.arange(NCORES), NPOOL)[None, :]

    K = int(K)
    sel = np.argpartition(-av, TOPW - 1, axis=1)[:, :TOPW]  # [B, TOPW]
    selc0 = np.take_along_axis(np.broadcast_to(col0, av.shape), sel, 1)
    selst = np.take_along_axis(np.broadcast_to(wstep, av.shape), sel, 1)
    selco = np.take_along_axis(np.broadcast_to(core_of, av.shape), sel, 1)
    cols = selc0[:, :, None] + selst[:, :, None] * np.arange(RW)[None, None, :]
    cols = cols.reshape(B, TOPW * RW)
    cores = np.repeat(selco, RW, axis=1)
    cand = orig[cores, cols]  # [B, TOPW*RW] original X row or -1
    invalid = cand < 0
    cand = np.where(invalid, 0, cand)

    qs = np.asarray(queries, np.float64)
    Xf = np.asarray(X, np.float64)
    CB = 64
    top = np.empty((B, K), np.int64)
    for i in range(0, B, CB):
        j = min(i + CB, B)
        Xc = Xf[cand[i:j].reshape(-1)].reshape(j - i, -1, D)
        d2 = ((Xc - qs[i:j, None, :]) ** 2).sum(-1)
        d2 += invalid[i:j] * 1e30
        order = np.argsort(d2, axis=1, kind="stable")[:, :K]
        top[i:j] = np.take_along_axis(cand[i:j], order, 1)

    labels = np.asarray(Y)[top].astype(np.float32)
    votes = labels.mean(1)
    out = np.zeros((B, 2), np.float32)
    out[:, 0] = votes
    return out


def kernel(queries, X, Y, K):
    res = _run_device(queries, X)
    return _merge(queries, X, Y, K, res)
